# revision 1
# baseline (speedup 1.0000x reference)
"""MetaLA decoder layer on 8 trn2 NeuronCores — v4 (own-tokens + state handoff).

Sharding: core = (batch b, sequence half). Each core processes ONLY its 1024
own tokens; the GLA state after the first half is handed to the second-half
core via an AllGather over core pairs (128KB), folded into the o-scan as a
decayed initial state.

v4: conv on PE (diag weights), LN1 affine on DVE (fused 2-scalar), k-outer
matmul order (stationary reuse halves LDWEIGHTS), collective issued before
the aug/intra-attention block (latency hidden), lean SBUF pools.
"""

import numpy as np
import ml_dtypes

import concourse.bass as bass
import concourse.mybir as mybir
import concourse.tile as tile
from concourse import bacc
from concourse.bass_utils import run_bass_kernel_spmd
from concourse.masks import make_identity, make_upper_triangular

F32 = mybir.dt.float32
BF16 = mybir.dt.bfloat16
AF = mybir.ActivationFunctionType
OP = mybir.AluOpType

P = 128
L, B, D = 2048, 4, 1024
H = 16
DK = 512
GLU_D = 2816
NORM = 16.0
EPS = 1e-5
KD = D // P
MQ = DK // P
NG = 4
CC = 128
NCH = 8
MG = GLU_D // P
TOWN = 1024
NTOK = TOWN + CC


def _r(x):
    return np.ascontiguousarray(x)


def _bf(x):
    return np.ascontiguousarray(x.astype(ml_dtypes.bfloat16))


def prep_host_inputs(inputs):
    f32 = lambda k: np.asarray(inputs[k], np.float32)
    tn_w, tn_b = f32("tn_w"), f32("tn_b")
    cn_w, cn_b = f32("cn_w"), f32("cn_b")
    conv = f32("conv_w")
    convw = conv * tn_w[:, None]
    convb = tn_b * conv.sum(1)
    l1 = f32("l1_w") * cn_w[:, None]
    l2 = f32("l2_w") * cn_w[:, None]
    b1 = cn_b @ f32("l1_w")
    b2 = cn_b @ f32("l2_w")

    def tile_kxm(w, m_tiles):
        d_in, m = w.shape
        kd = d_in // P
        return w.reshape(kd, P, m_tiles, m // m_tiles).transpose(1, 0, 2, 3)

    shared = {
        "wq": _bf(tile_kxm(f32("q_w"), MQ).reshape(P, -1)),
        "wkg": _bf(tile_kxm(f32("kg_w"), MQ).reshape(P, -1)),
        "wv": _bf(f32("v_w").reshape(KD, P, D).transpose(1, 0, 2).reshape(P, -1)),
        "wg": _bf(f32("g_w").reshape(KD, P, D).transpose(1, 0, 2).reshape(P, -1)),
        "wout": _bf(f32("out_w").reshape(KD, P, D).transpose(1, 0, 2).reshape(P, -1)),
        "wl1": _bf(l1.reshape(KD, P, MG, P).transpose(1, 2, 0, 3).reshape(P, -1)),
        "wl2": _bf(l2.reshape(KD, P, MG, P).transpose(1, 2, 0, 3).reshape(P, -1)),
        "wl3": _bf(f32("l3_w").reshape(MG, P, D).transpose(1, 0, 2).reshape(P, -1)),
        "convw": _r(convw.reshape(KD, P, 4).transpose(1, 0, 2).reshape(P, -1)),
        "convb": _r(convb.reshape(KD, P).T),
        "aug": _r(f32("aug_balance").reshape(MQ, P).T),
        "b1": _r(b1.reshape(MG, P).T),
        "b2": _r(b2.reshape(MG, P).T),
    }
    x = np.asarray(inputs["x"], np.float32)

    def per_core(core):
        b, half = core // 2, core % 2
        if half == 0:
            x_seq = np.concatenate(
                [np.zeros((CC, D), np.float32), x[0:TOWN, b]], axis=0)
        else:
            x_seq = _r(x[TOWN - CC:L, b, :])
        flag = np.full((P, 1), float(half), np.float32)
        return {"x_seq": x_seq, "flag": flag, **shared}

    return per_core


def build_nc():
    nc = bacc.Bacc("TRN2", target_bir_lowering=False, debug=False)
    dt_in = {
        "x_seq": ([NTOK, D], F32),
        "flag": ([P, 1], F32),
        "wq": ([P, KD * MQ * P], BF16),
        "wkg": ([P, KD * MQ * P], BF16),
        "wv": ([P, KD * D], BF16),
        "wg": ([P, KD * D], BF16),
        "wout": ([P, KD * D], BF16),
        "wl1": ([P, MG * KD * P], BF16),
        "wl2": ([P, MG * KD * P], BF16),
        "wl3": ([P, MG * D], BF16),
        "convw": ([P, KD * 4], F32),
        "convb": ([P, KD], F32),
        "aug": ([P, MQ], F32),
        "b1": ([P, MG], F32),
        "b2": ([P, MG], F32),
    }
    dr = {k: nc.dram_tensor(k, shp, dt, kind="ExternalInput")
          for k, (shp, dt) in dt_in.items()}
    out_d = nc.dram_tensor("out", [TOWN, D], F32, kind="ExternalOutput")
    x1_d = nc.dram_tensor("x1buf", [TOWN, D], F32)

    with tile.TileContext(nc) as tc:
        _emit(nc, tc, dr, out_d, x1_d)
    nc.compile()
    return nc


def _emit(nc, tc, dr, out_d, x1_d):
    import contextlib
    ctx = contextlib.ExitStack()
    with ctx:
        sing = ctx.enter_context(tc.tile_pool(name="sing", bufs=1))
        dram = ctx.enter_context(tc.tile_pool(name="dram", bufs=1, space="DRAM"))
        work = ctx.enter_context(tc.tile_pool(name="work", bufs=1))

        convw_sb = sing.tile([P, KD, 4], F32)
        nc.sync.dma_start(out=convw_sb,
                          in_=dr["convw"].ap().rearrange("p (k i) -> p k i", k=KD))
        convb_sb = sing.tile([P, KD], F32)
        nc.sync.dma_start(out=convb_sb, in_=dr["convb"].ap())
        aug_sb = sing.tile([P, MQ], F32)
        nc.sync.dma_start(out=aug_sb, in_=dr["aug"].ap())
        b1_sb = sing.tile([P, MG], F32)
        nc.sync.dma_start(out=b1_sb, in_=dr["b1"].ap())
        b2_sb = sing.tile([P, MG], F32)
        nc.sync.dma_start(out=b2_sb, in_=dr["b2"].ap())
        flag_sb = sing.tile([P, 1], F32)
        nc.sync.dma_start(out=flag_sb, in_=dr["flag"].ap())

        identb = sing.tile([P, P], BF16)
        make_identity(nc, identb)
        maskc = sing.tile([P, P], BF16)
        make_upper_triangular(nc, maskc, val=1.0, diag=True)
        ind4 = sing.tile([P, 4], BF16)
        nc.vector.memset(ind4, 0.0)
        for j in range(4):
            nc.vector.memset(ind4[32 * j:32 * j + 32, j:j + 1], 1.0)
        epst = sing.tile([P, 1], F32)
        nc.vector.memset(epst, EPS)

        wout_sb = sing.tile([P, KD, D], BF16)
        zT = sing.tile([P, KD, TOWN], BF16)   # GLU input, survives to P9

        x_ap = dr["x_seq"].ap()

        # long-lived working set
        kt = work.tile([P, MQ, TOWN], BF16)
        qt = work.tile([P, MQ, TOWN], BF16)
        qk = work.tile([P, MQ, TOWN], BF16)
        dCt = work.tile([P, MQ, NCH], F32)
        dcum = work.tile([P, MQ, NCH], F32)
        v_t = work.tile([P, NCH, D], BF16)
        g_t = work.tile([P, NCH, D], BF16)
        sg = work.tile([P, NCH, D], BF16)
        s_hist = work.tile([P, NCH, MQ, 64], BF16)
        s_in = work.tile([P, MQ, 64], F32)
        NPRE = 2
        ptm = work.tile([P, NPRE, NG, 4, P], BF16)

        with tc.tile_pool(name="py", bufs=1) as py:
            yT = py.tile([P, KD, TOWN], BF16)

            # ========== P1: LN1 + transpose (9 chunks incl halo) ============
            # ========== P2: conv (PE, diag weights) + silu -> yT =============
            with tc.tile_pool(name="p12", bufs=1) as ep, \
                 tc.tile_pool(name="p12ps", bufs=1, space="PSUM") as pp:
                hT = ep.tile([P, KD, NTOK], BF16)
                for t in range(NCH + 1):
                    x_t = ep.tile([P, D], F32, tag="x_nat", bufs=2, name=f"x_{t}")
                    nc.sync.dma_start(out=x_t, in_=x_ap[t * CC:(t + 1) * CC, :])
                    bnst = ep.tile([P, 2, 6], F32, tag="bnst", bufs=2,
                                   name=f"bn_{t}")
                    nc.vector.bn_stats(out=bnst[:, 0, :], in_=x_t[:, 0:512])
                    nc.vector.bn_stats(out=bnst[:, 1, :], in_=x_t[:, 512:1024])
                    mv = ep.tile([P, 2], F32, tag="mv", bufs=2, name=f"mv_{t}")
                    nc.vector.bn_aggr(out=mv, in_=bnst)
                    srt = ep.tile([P, 1], F32, tag="srt", bufs=2, name=f"sq_{t}")
                    nc.scalar.activation(out=srt, in_=mv[:, 1:2], func=AF.Sqrt,
                                         bias=epst[:, 0:1], scale=1.0)
                    rstd = ep.tile([P, 1], F32, tag="rstd", bufs=2, name=f"rs_{t}")
                    nc.vector.reciprocal(out=rstd, in_=srt)
                    nmrs = ep.tile([P, 1], F32, tag="nmrs", bufs=2, name=f"nm_{t}")
                    nc.vector.scalar_tensor_tensor(out=nmrs, in0=mv[:, 0:1],
                                                   scalar=-1.0, in1=rstd,
                                                   op0=OP.mult, op1=OP.mult)
                    h = ep.tile([P, D], BF16, tag="h", bufs=2, name=f"h_{t}")
                    nc.vector.tensor_scalar(out=h, in0=x_t,
                                            scalar1=rstd[:, 0:1],
                                            scalar2=nmrs[:, 0:1],
                                            op0=OP.mult, op1=OP.add)
                    ptr = pp.tile([P, D], BF16, tag="ptr", bufs=2, name=f"ptr_{t}")
                    for dt_i in range(KD):
                        nc.tensor.transpose(ptr[:, dt_i * P:(dt_i + 1) * P],
                                            h[:, dt_i * P:(dt_i + 1) * P], identb)
                    nc.scalar.copy(out=hT[:, :, t * CC:(t + 1) * CC],
                                   in_=ptr.rearrange("p (k c) -> p k c", k=KD))

                nc.sync.dma_start(
                    out=wout_sb,
                    in_=dr["wout"].ap().rearrange("p (k n) -> p k n", k=KD))

                # conv on PE: diagonal weight tiles, 4 taps accumulate in PSUM
                cdiag = ep.tile([P, KD, 4, P], BF16)
                for dt_i in range(KD):
                    for i in range(4):
                        nc.vector.tensor_scalar(out=cdiag[:, dt_i, i, :],
                                                in0=identb,
                                                scalar1=convw_sb[:, dt_i, i:i + 1],
                                                scalar2=None, op0=OP.mult)
                for dt_i in range(KD):
                    pc = pp.tile([P, TOWN], F32, tag="pc", bufs=2,
                                 name=f"pc_{dt_i}")
                    for n in range(2):
                        nsl = slice(n * 512, (n + 1) * 512)
                        for i in range(4):
                            nc.tensor.matmul(
                                pc[:, nsl], cdiag[:, dt_i, i, :],
                                hT[:, dt_i, 125 + i + n * 512:
                                   125 + i + n * 512 + 512],
                                start=(i == 0), stop=(i == 3))
                    nc.scalar.activation(out=yT[:, dt_i, :], in_=pc, func=AF.Silu,
                                         bias=convb_sb[:, dt_i:dt_i + 1], scale=1.0)

            # ========== P3: kg/q projections + gating ========================
            with tc.tile_pool(name="p3", bufs=1) as e3, \
                 tc.tile_pool(name="p3ps", bufs=1, space="PSUM") as pp3:
                wq_sb = e3.tile([P, KD, MQ, P], BF16)
                nc.sync.dma_start(
                    out=wq_sb,
                    in_=dr["wq"].ap().rearrange("p (k m c) -> p k m c", k=KD, m=MQ))
                wkg_sb = e3.tile([P, KD, MQ, P], BF16)
                nc.sync.dma_start(
                    out=wkg_sb,
                    in_=dr["wkg"].ap().rearrange("p (k m c) -> p k m c", k=KD, m=MQ))

                for g0 in range(0, NG, 2):
                    kgg, qTb, t0s, sps, As, Bxs = {}, {}, {}, {}, {}, {}
                    for g in (g0, g0 + 1):
                        ps = pp3.tile([P, TOWN], F32, tag="pkg", bufs=2,
                                      name=f"pkg_{g}")
                        for k in range(KD):
                            for n in range(2):
                                nsl = slice(n * 512, (n + 1) * 512)
                                nc.tensor.matmul(ps[:, nsl], wkg_sb[:, k, g, :],
                                                 yT[:, k, nsl], start=(k == 0),
                                                 stop=(k == KD - 1))
                        kgg[g] = ps
                        ps2 = pp3.tile([P, TOWN], F32, tag="pq", bufs=2,
                                       name=f"pq_{g}")
                        for k in range(KD):
                            for n in range(2):
                                nsl = slice(n * 512, (n + 1) * 512)
                                nc.tensor.matmul(ps2[:, nsl], wq_sb[:, k, g, :],
                                                 yT[:, k, nsl], start=(k == 0),
                                                 stop=(k == KD - 1))
                        q_g = e3.tile([P, TOWN], BF16, tag="qTb", bufs=2,
                                      name=f"qTb_{g}")
                        nc.scalar.copy(out=q_g, in_=ps2)
                        qTb[g] = q_g
                    for g in (g0, g0 + 1):
                        t0 = e3.tile([P, TOWN], F32, tag="t0", bufs=2,
                                     name=f"t0_{g}")
                        nc.scalar.activation(out=t0, in_=kgg[g], func=AF.Exp,
                                             scale=-1.0)
                        t0s[g] = t0
                    for g in (g0, g0 + 1):
                        sp = e3.tile([P, TOWN], F32, tag="sp", bufs=2,
                                     name=f"sp_{g}")
                        nc.scalar.activation(out=sp, in_=t0s[g], func=AF.Ln,
                                             bias=1.0, scale=1.0)
                        sps[g] = sp
                    for g in (g0, g0 + 1):
                        A = e3.tile([P, TOWN], F32, tag="A", bufs=2, name=f"A_{g}")
                        nc.vector.tensor_tensor_scan(out=A, data0=sps[g],
                                                     data1=sps[g], initial=0.0,
                                                     op0=OP.add, op1=OP.bypass)
                        As[g] = A
                        bx = e3.tile([P, NCH + 1], F32, tag="bx", bufs=2,
                                     name=f"bx_{g}")
                        nc.vector.memset(bx[:, 0:1], 0.0)
                        nc.vector.tensor_scalar(
                            out=bx[:, 1:NCH + 1],
                            in0=A.rearrange("p (c t) -> p c t", c=NCH)[:, :, CC - 1],
                            scalar1=1.0 / NORM, scalar2=None, op0=OP.mult)
                        Bxs[g] = bx
                        nbx = e3.tile([P, NCH], F32, tag="nbx", bufs=2,
                                      name=f"nbx_{g}")
                        nc.vector.tensor_scalar(out=nbx, in0=bx[:, 0:NCH],
                                                scalar1=-NORM, scalar2=None,
                                                op0=OP.mult)
                        for c in range(1, NCH):
                            nc.vector.tensor_scalar(
                                out=A[:, c * CC:(c + 1) * CC],
                                in0=A[:, c * CC:(c + 1) * CC],
                                scalar1=nbx[:, c:c + 1], scalar2=None, op0=OP.add)
                    for g in (g0, g0 + 1):
                        ek = e3.tile([P, TOWN], F32, tag="ek", bufs=1,
                                     name=f"ek_{g}")
                        nc.scalar.activation(out=ek, in_=sps[g], func=AF.Exp,
                                             scale=-1.0 / NORM)
                        eA = e3.tile([P, TOWN], BF16, tag="eA", bufs=1,
                                     name=f"eA_{g}")
                        nc.scalar.activation(out=eA, in_=As[g], func=AF.Exp,
                                             scale=-1.0 / NORM)
                        erA = e3.tile([P, TOWN], BF16, tag="erA", bufs=1,
                                      name=f"erA_{g}")
                        nc.scalar.activation(out=erA, in_=As[g], func=AF.Exp,
                                             scale=1.0 / NORM)
                        nc.scalar.activation(
                            out=dCt[:, g, :],
                            in_=As[g].rearrange("p (c t) -> p c t",
                                                c=NCH)[:, :, CC - 1],
                            func=AF.Exp, scale=-1.0 / NORM)
                        nc.scalar.activation(out=dcum[:, g, :],
                                             in_=Bxs[g][:, 0:NCH],
                                             func=AF.Exp, scale=-1.0)
                        k_ = e3.tile([P, TOWN], BF16, tag="k_", bufs=1,
                                     name=f"k__{g}")
                        nc.vector.tensor_scalar(out=k_, in0=ek, scalar1=-1.0,
                                                scalar2=1.0, op0=OP.mult,
                                                op1=OP.add)
                        nc.vector.tensor_tensor(out=kt[:, g, :], in0=k_, in1=erA,
                                                op=OP.mult)
                        nc.vector.tensor_tensor(out=qt[:, g, :], in0=qTb[g],
                                                in1=eA, op=OP.mult)
                        nc.vector.scalar_tensor_tensor(
                            out=qk[:, g, :], in0=qTb[g],
                            scalar=aug_sb[:, g:g + 1], in1=k_,
                            op0=OP.mult, op1=OP.mult)

            # ========== P4: v/g projections ==================================
            with tc.tile_pool(name="p4", bufs=1) as e4, \
                 tc.tile_pool(name="p4ps", bufs=1, space="PSUM") as pp4:
                wv_sb = e4.tile([P, KD, D], BF16)
                nc.sync.dma_start(
                    out=wv_sb, in_=dr["wv"].ap().rearrange("p (k n) -> p k n", k=KD))
                wg_sb = e4.tile([P, KD, D], BF16)
                nc.sync.dma_start(
                    out=wg_sb, in_=dr["wg"].ap().rearrange("p (k n) -> p k n", k=KD))
                for c in range(NCH):
                    csl = slice(c * CC, (c + 1) * CC)
                    psv = pp4.tile([P, D], F32, tag="psv", bufs=2, name=f"psv_{c}")
                    for k in range(KD):
                        for n in range(2):
                            nsl = slice(n * 512, (n + 1) * 512)
                            nc.tensor.matmul(psv[:, nsl], yT[:, k, csl],
                                             wv_sb[:, k, nsl], start=(k == 0),
                                             stop=(k == KD - 1))
                    nc.scalar.copy(out=v_t[:, c, :], in_=psv)
                    psg = pp4.tile([P, D], F32, tag="psg", bufs=2, name=f"psg_{c}")
                    for k in range(KD):
                        for n in range(2):
                            nsl = slice(n * 512, (n + 1) * 512)
                            nc.tensor.matmul(psg[:, nsl], yT[:, k, csl],
                                             wg_sb[:, k, nsl], start=(k == 0),
                                             stop=(k == KD - 1))
                    nc.scalar.activation(out=g_t[:, c, :], in_=psg, func=AF.Silu)
        # py closed: yT freed

        # ========== P5: state scan -> handoff -> aug + intra-attn ===========
        with tc.tile_pool(name="p5", bufs=1) as e5, \
             tc.tile_pool(name="p5ps", bufs=1, space="PSUM") as pp5:
            s_st = e5.tile([P, MQ, 64], F32)
            nc.vector.memset(s_st, 0.0)
            for c in range(NCH):
                csl = slice(c * CC, (c + 1) * CC)
                nc.vector.tensor_copy(out=s_hist[:, c, :, :], in_=s_st)
                for g in range(NG):
                    ktmp = e5.tile([P, P], BF16, tag="ktmp", bufs=4,
                                   name=f"ktmp_{c}_{g}")
                    nc.vector.tensor_scalar(out=ktmp, in0=kt[:, g, csl],
                                            scalar1=dCt[:, g, c:c + 1],
                                            scalar2=None, op0=OP.mult)
                    ktr = pp5.tile([P, P], BF16, tag="ktr", bufs=2,
                                   name=f"ktr_{c}_{g}")
                    nc.tensor.transpose(ktr, ktmp, identb)
                    kn = e5.tile([P, P], BF16, tag="kn", bufs=4, name=f"kn_{c}_{g}")
                    nc.scalar.copy(out=kn, in_=ktr)
                    dsp = pp5.tile([P, 64], F32, tag="dsp", bufs=2,
                                   name=f"dsp_{c}_{g}")
                    for hh in range(4):
                        hd = 4 * g + hh
                        hs = slice(32 * hh, 32 * hh + 32)
                        nc.tensor.matmul(dsp[hs, :], kn[:, hs],
                                         v_t[:, c, 64 * hd:64 * hd + 64],
                                         start=True, stop=True,
                                         tile_position=(0, 32 * hh))
                    nc.vector.scalar_tensor_tensor(out=s_st[:, g, :],
                                                   in0=s_st[:, g, :],
                                                   scalar=dCt[:, g, c:c + 1],
                                                   in1=dsp, op0=OP.mult,
                                                   op1=OP.add)
            # state handoff (AllGather over core pairs)
            s_out_b = dram.tile([P, MQ * 64], F32)
            s_gth_b = dram.tile([2, P, MQ * 64], F32)
            nc.gpsimd.dma_start(out=s_out_b,
                                in_=s_st.rearrange("p g c -> p (g c)"))
            nc.gpsimd.collective_compute(
                "AllGather", OP.bypass,
                replica_groups=[[0, 1], [2, 3], [4, 5], [6, 7]],
                ins=[s_out_b.opt()], outs=[s_gth_b.opt()],
            )
            # aug term (independent of state; hides collective latency)
            for c in range(NCH):
                csl = slice(c * CC, (c + 1) * CC)
                pa = pp5.tile([P, H], F32, tag="psaug", bufs=1, name=f"pa_{c}")
                for g in range(NG):
                    nc.tensor.matmul(pa[:, 4 * g:4 * g + 4], qk[:, g, csl], ind4,
                                     start=True, stop=True)
                aug_t = e5.tile([P, H], BF16, tag="aug_t", bufs=2, name=f"at_{c}")
                nc.vector.tensor_copy(out=aug_t, in_=pa)
                augv = e5.tile([P, D], BF16, tag="augv", bufs=2, name=f"av_{c}")
                nc.vector.tensor_tensor(
                    out=augv.rearrange("p (h c) -> p h c", h=H),
                    in0=v_t[:, c, :].rearrange("p (h c) -> p h c", h=H),
                    in1=aug_t[:, :, None].broadcast_to((P, H, 64)), op=OP.mult)
                nc.scalar.activation(out=sg[:, c, :], in_=augv, func=AF.Sigmoid)
            # masked intra-chunk attention, first NPRE chunks (hides CC)
            for c in range(NPRE):
                csl = slice(c * CC, (c + 1) * CC)
                for g in range(NG):
                    for hh in range(4):
                        hs = slice(32 * hh, 32 * hh + 32)
                        pt_ps = pp5.tile([P, P], F32, tag="ptps", bufs=3,
                                         name=f"ptps_{c}_{g}_{hh}")
                        nc.tensor.matmul(pt_ps, kt[hs, g, csl], qt[hs, g, csl],
                                         start=True, stop=True,
                                         tile_position=(32 * hh, 0))
                        nc.vector.tensor_tensor(out=ptm[:, c, g, hh, :],
                                                in0=pt_ps, in1=maskc, op=OP.mult)
            s_in_raw = e5.tile([P, MQ * 64], F32)
            nc.gpsimd.dma_start(out=s_in_raw, in_=s_gth_b[0])
            nc.vector.tensor_scalar(out=s_in.rearrange("p g c -> p (g c)"),
                                    in0=s_in_raw, scalar1=flag_sb[:, 0:1],
                                    scalar2=None, op0=OP.mult)

        # ========== P7: o assembly + gn + gate + out proj + LN2 + zT ========
        with tc.tile_pool(name="p7", bufs=1) as e7, \
             tc.tile_pool(name="p7ps", bufs=1, space="PSUM") as pp7:
            for c in range(NCH):
                csl = slice(c * CC, (c + 1) * CC)
                se1 = e7.tile([P, MQ, 64], F32, tag="se1", bufs=3, name=f"se1_{c}")
                nc.vector.tensor_tensor(
                    out=se1, in0=s_in,
                    in1=dcum[:, :, c, None].broadcast_to((P, MQ, 64)), op=OP.mult)
                s_eff = e7.tile([P, MQ, 64], BF16, tag="seff", bufs=3,
                                name=f"seff_{c}")
                nc.vector.tensor_tensor(out=s_eff, in0=se1,
                                        in1=s_hist[:, c, :, :], op=OP.add)
                o_ps = pp7.tile([P, D], F32, tag="pso", bufs=2, name=f"ops_{c}")
                if c < NPRE:
                    ptmc = ptm[:, c]
                else:
                    ptmc = e7.tile([P, NG, 4, P], BF16, tag="ptmc", bufs=2,
                                   name=f"ptmc_{c}")
                    for g in range(NG):
                        for hh in range(4):
                            hs = slice(32 * hh, 32 * hh + 32)
                            pt_ps = pp7.tile([P, P], F32, tag="ptps", bufs=2,
                                             name=f"ptps_{c}_{g}_{hh}")
                            nc.tensor.matmul(pt_ps, kt[hs, g, csl],
                                             qt[hs, g, csl],
                                             start=True, stop=True,
                                             tile_position=(32 * hh, 0))
                            nc.vector.tensor_tensor(out=ptmc[:, g, hh, :],
                                                    in0=pt_ps, in1=maskc,
                                                    op=OP.mult)
                for g in range(NG):
                    for hh in range(4):
                        hd = 4 * g + hh
                        hs = slice(32 * hh, 32 * hh + 32)
                        osl = slice(64 * hd, 64 * hd + 64)
                        nc.tensor.matmul(o_ps[:, osl], qt[hs, g, csl],
                                         s_eff[hs, g, :], start=True, stop=False,
                                         tile_position=(32 * hh, 0))
                        nc.tensor.matmul(o_ps[:, osl], ptmc[:, g, hh, :],
                                         v_t[:, c, osl], start=False, stop=True)
                o_sb = e7.tile([P, D], BF16, tag="o_sb", bufs=3, name=f"osb_{c}")
                nc.vector.tensor_tensor(out=o_sb, in0=o_ps, in1=sg[:, c, :],
                                        op=OP.add)
                o_h = o_sb.rearrange("p (h c) -> p h c", h=H)
                osq = e7.tile([P, D], BF16, tag="osq", bufs=3, name=f"osq_{c}")
                nc.vector.tensor_tensor(out=osq, in0=o_sb, in1=o_sb, op=OP.mult)
                s1 = e7.tile([P, H], F32, tag="s1", bufs=2, name=f"s1_{c}")
                nc.vector.tensor_reduce(out=s1, in_=o_h,
                                        axis=mybir.AxisListType.X, op=OP.add)
                s2 = e7.tile([P, H], F32, tag="s2", bufs=2, name=f"s2_{c}")
                nc.vector.tensor_reduce(out=s2,
                                        in_=osq.rearrange("p (h c) -> p h c", h=H),
                                        axis=mybir.AxisListType.X, op=OP.add)
                mo = e7.tile([P, H], F32, tag="mo", bufs=2, name=f"mo_{c}")
                nc.vector.tensor_scalar(out=mo, in0=s1, scalar1=1.0 / 64.0,
                                        scalar2=None, op0=OP.mult)
                mo2 = e7.tile([P, H], F32, tag="mo2", bufs=2, name=f"mo2_{c}")
                nc.vector.tensor_tensor(out=mo2, in0=mo, in1=mo, op=OP.mult)
                varo = e7.tile([P, H], F32, tag="varo", bufs=2, name=f"varo_{c}")
                nc.vector.scalar_tensor_tensor(out=varo, in0=s2, scalar=1.0 / 64.0,
                                               in1=mo2, op0=OP.mult,
                                               op1=OP.subtract)
                srto = e7.tile([P, H], F32, tag="srto", bufs=2, name=f"sqo_{c}")
                nc.scalar.activation(out=srto, in_=varo, func=AF.Sqrt,
                                     bias=epst[:, 0:1], scale=1.0)
                rstdo = e7.tile([P, H], F32, tag="rstdo", bufs=2, name=f"rso_{c}")
                nc.vector.reciprocal(out=rstdo, in_=srto)
                nmo = e7.tile([P, H], BF16, tag="nmo", bufs=2, name=f"nmo_{c}")
                nc.vector.scalar_tensor_tensor(out=nmo, in0=mo, scalar=-1.0,
                                               in1=rstdo, op0=OP.mult, op1=OP.mult)
                on1 = e7.tile([P, D], BF16, tag="on1", bufs=3, name=f"on1_{c}")
                nc.vector.tensor_tensor(
                    out=on1.rearrange("p (h c) -> p h c", h=H), in0=o_h,
                    in1=rstdo[:, :, None].broadcast_to((P, H, 64)), op=OP.mult)
                on2 = e7.tile([P, D], BF16, tag="on2", bufs=3, name=f"on2_{c}")
                nc.vector.tensor_tensor(
                    out=on2.rearrange("p (h c) -> p h c", h=H),
                    in0=on1.rearrange("p (h c) -> p h c", h=H),
                    in1=nmo[:, :, None].broadcast_to((P, H, 64)), op=OP.add)
                og = e7.tile([P, D], BF16, tag="og", bufs=3, name=f"og_{c}")
                nc.vector.tensor_tensor(out=og, in0=on2, in1=g_t[:, c, :],
                                        op=OP.mult)
                ogt_ps = pp7.tile([P, D], BF16, tag="trps", bufs=2, name=f"ogt_{c}")
                for dt_i in range(KD):
                    nc.tensor.transpose(ogt_ps[:, dt_i * P:(dt_i + 1) * P],
                                        og[:, dt_i * P:(dt_i + 1) * P], identb)
                ogT = e7.tile([P, KD, P], BF16, tag="ogT", bufs=3, name=f"ogT_{c}")
                nc.scalar.copy(out=ogT,
                               in_=ogt_ps.rearrange("p (k c) -> p k c", k=KD))
                out_ps = pp7.tile([P, D], F32, tag="pso", bufs=2, name=f"op_{c}")
                for k in range(KD):
                    for n in range(2):
                        nsl = slice(n * 512, (n + 1) * 512)
                        nc.tensor.matmul(out_ps[:, nsl], ogT[:, k, :],
                                         wout_sb[:, k, nsl], start=(k == 0),
                                         stop=(k == KD - 1))
                x_res = e7.tile([P, D], F32, tag="x_res", bufs=2, name=f"xr_{c}")
                nc.sync.dma_start(out=x_res,
                                  in_=x_ap[CC + c * CC:CC + (c + 1) * CC, :])
                x1 = e7.tile([P, D], F32, tag="x1", bufs=2, name=f"x1_{c}")
                nc.vector.scalar_tensor_tensor(out=x1, in0=out_ps, scalar=1.0,
                                               in1=x_res, op0=OP.bypass,
                                               op1=OP.add)
                nc.sync.dma_start(out=x1_d.ap()[c * CC:(c + 1) * CC, :], in_=x1)
                # LN2 (sqrt+recip — single ACT table era in P7)
                bn2 = e7.tile([P, 2, 6], F32, tag="bn2", bufs=2, name=f"bn2_{c}")
                nc.vector.bn_stats(out=bn2[:, 0, :], in_=x1[:, 0:512])
                nc.vector.bn_stats(out=bn2[:, 1, :], in_=x1[:, 512:1024])
                mv2 = e7.tile([P, 2], F32, tag="mv2", bufs=2, name=f"mv2_{c}")
                nc.vector.bn_aggr(out=mv2, in_=bn2)
                srt2 = e7.tile([P, 1], F32, tag="srt2", bufs=2, name=f"sq2_{c}")
                nc.scalar.activation(out=srt2, in_=mv2[:, 1:2], func=AF.Sqrt,
                                     bias=epst[:, 0:1], scale=1.0)
                rstd2 = e7.tile([P, 1], F32, tag="rstd2", bufs=2, name=f"rs2_{c}")
                nc.vector.reciprocal(out=rstd2, in_=srt2)
                nmrs2 = e7.tile([P, 1], F32, tag="nmrs2", bufs=2, name=f"nm2_{c}")
                nc.vector.scalar_tensor_tensor(out=nmrs2, in0=mv2[:, 0:1],
                                               scalar=-1.0, in1=rstd2,
                                               op0=OP.mult, op1=OP.mult)
                z = e7.tile([P, D], BF16, tag="z", bufs=3, name=f"z_{c}")
                nc.scalar.activation(out=z, in_=x1, func=AF.Identity,
                                     bias=nmrs2[:, 0:1], scale=rstd2[:, 0:1])
                ztr_ps = pp7.tile([P, D], BF16, tag="trps", bufs=2, name=f"ztr_{c}")
                for dt_i in range(KD):
                    nc.tensor.transpose(ztr_ps[:, dt_i * P:(dt_i + 1) * P],
                                        z[:, dt_i * P:(dt_i + 1) * P], identb)
                nc.scalar.copy(out=zT[:, :, csl],
                               in_=ztr_ps.rearrange("p (k c) -> p k c", k=KD))

        # ========== P9: GLU ==========
        wl1_v = dr["wl1"].ap().rearrange("p (m k c) -> p m k c", m=MG, k=KD)
        wl2_v = dr["wl2"].ap().rearrange("p (m k c) -> p m k c", m=MG, k=KD)
        wl3_v = dr["wl3"].ap().rearrange("p (k n) -> p k n", k=MG)
        with tc.tile_pool(name="gl", bufs=1) as gl:
            gluT = gl.tile([P, MG, TOWN], BF16)
            with tc.tile_pool(name="glps", bufs=1, space="PSUM") as glps:
                for m in range(MG):
                    l1m = gl.tile([P, KD, P], BF16, tag="l1m", bufs=3,
                                  name=f"l1m_{m}")
                    nc.sync.dma_start(out=l1m, in_=wl1_v[:, m, :, :])
                    l2m = gl.tile([P, KD, P], BF16, tag="l2m", bufs=3,
                                  name=f"l2m_{m}")
                    nc.sync.dma_start(out=l2m, in_=wl2_v[:, m, :, :])
                    g1p = glps.tile([P, TOWN], F32, tag="psg1", bufs=2,
                                    name=f"g1p_{m}")
                    for k in range(KD):
                        for n in range(2):
                            nsl = slice(n * 512, (n + 1) * 512)
                            nc.tensor.matmul(g1p[:, nsl], l1m[:, k, :],
                                             zT[:, k, nsl], start=(k == 0),
                                             stop=(k == KD - 1))
                    g1s = gl.tile([P, TOWN], BF16, tag="g1s", bufs=2,
                                  name=f"g1s_{m}")
                    nc.scalar.activation(out=g1s, in_=g1p, func=AF.Silu,
                                         bias=b1_sb[:, m:m + 1], scale=1.0)
                    g2p = glps.tile([P, TOWN], F32, tag="psg2", bufs=2,
                                    name=f"g2p_{m}")
                    for k in range(KD):
                        for n in range(2):
                            nsl = slice(n * 512, (n + 1) * 512)
                            nc.tensor.matmul(g2p[:, nsl], l2m[:, k, :],
                                             zT[:, k, nsl], start=(k == 0),
                                             stop=(k == KD - 1))
                    nc.vector.scalar_tensor_tensor(out=gluT[:, m, :], in0=g2p,
                                                   scalar=b2_sb[:, m:m + 1],
                                                   in1=g1s, op0=OP.add,
                                                   op1=OP.mult)

            with tc.tile_pool(name="g3", bufs=1) as g3, \
                 tc.tile_pool(name="g3ps", bufs=1, space="PSUM") as g3ps:
                for t in range(2):
                    psf = [g3ps.tile([P, 512], F32, tag="psf", bufs=8,
                                     name=f"psf_{t}_{i}") for i in range(8)]
                    for k in range(MG):
                        l3k = g3.tile([P, D], BF16, tag="l3k", bufs=3,
                                      name=f"l3k_{t}_{k}")
                        nc.sync.dma_start(out=l3k, in_=wl3_v[:, k, :])
                        for cc in range(4):
                            for n in range(2):
                                nc.tensor.matmul(
                                    psf[cc * 2 + n],
                                    gluT[:, k, t * 512 + cc * P:
                                         t * 512 + (cc + 1) * P],
                                    l3k[:, n * 512:(n + 1) * 512],
                                    start=(k == 0), stop=(k == MG - 1))
                    for cc in range(4):
                        trow = t * 512 + cc * CC
                        x1c = g3.tile([P, D], F32, tag="x1c", bufs=2,
                                      name=f"x1c_{t}_{cc}")
                        nc.sync.dma_start(out=x1c,
                                          in_=x1_d.ap()[trow:trow + CC, :])
                        outc = g3.tile([P, D], F32, tag="outc", bufs=2,
                                       name=f"outc_{t}_{cc}")
                        for n in range(2):
                            nc.vector.scalar_tensor_tensor(
                                out=outc[:, n * 512:(n + 1) * 512],
                                in0=psf[cc * 2 + n], scalar=1.0,
                                in1=x1c[:, n * 512:(n + 1) * 512],
                                op0=OP.bypass, op1=OP.add)
                        nc.sync.dma_start(out=out_d.ap()[trow:trow + CC, :],
                                          in_=outc)


_NC_CACHE = {}


def get_nc():
    if "nc" not in _NC_CACHE:
        _NC_CACHE["nc"] = build_nc()
    return _NC_CACHE["nc"]


def kernel(**inputs):
    nc = get_nc()
    per_core = prep_host_inputs(inputs)
    in_maps = [per_core(c) for c in range(8)]
    res = run_bass_kernel_spmd(nc, in_maps, core_ids=list(range(8)))
    out = np.zeros((L, B, D), np.float32)
    for c in range(8):
        b, half = c // 2, c % 2
        out[half * TOWN:(half + 1) * TOWN, b, :] = res.results[c]["out"]
    return out



# revision 30
# speedup vs baseline: 1.4886x; 1.4886x over previous
"""MetaLA decoder layer on 8 trn2 NeuronCores — v5 (fp8 DoubleRow).

Sharding: core = (batch b, sequence half); 1024 own tokens per core; GLA
state handed to the second-half core via AllGather over core pairs.

v5 over v4:
- All eight big projections (q/kg/v/g/out/l1/l2/l3) run in fp8e4m3 with
  DoubleRow perf mode (2 k-tiles per instruction, 0.5 cyc/row = 2x bf16).
  Scales: y*64, weights*32 (l2*8), og*256, glu*8, scores*32 (folded into
  the causal mask values), v*64 stored fp8. All descales fold into
  existing activation `scale` args or scalar slots - zero extra ops.
- v/g weights preloaded at P3 open (kills the 26us PE stall).
- v-projection interleaved with the state scan per chunk; the collective
  issues right after, hidden behind g-projection + aug + all-8-chunk
  score matmuls.
- P7 merged o-assembly/groupnorm loop with 2-chunk skew so PE small
  matmuls overlap the Vector groupnorm/LN2 chains; elementwise work
  spread across Vector/Pool/Scalar.
- Scores for 4 heads collected in one PSUM bank, single masked multiply.
- GLU: l3 weights resident (loaded once), x1 kept in SBUF (no DRAM
  round-trip), ACT-table thrash minimized by batching same-func stages.
"""

import numpy as np
import ml_dtypes

import concourse.bass as bass
import concourse.mybir as mybir
import concourse.tile as tile
from concourse import bacc
from concourse.bass_utils import run_bass_kernel_spmd
from concourse.masks import make_identity, make_upper_triangular

F32 = mybir.dt.float32
BF16 = mybir.dt.bfloat16
F8 = mybir.dt.float8e4
AF = mybir.ActivationFunctionType
OP = mybir.AluOpType
DRM = mybir.MatmulPerfMode.DoubleRow

P = 128
L, B, D = 2048, 4, 1024
H = 16
DK = 512
GLU_D = 2816
NORM = 16.0
EPS = 1e-5
KD = D // P          # 8 k-tiles over D
MQ = DK // P         # 4 q/k col tiles (= head groups)
NG = 4
CC = 128
NCH = 8
MG = GLU_D // P      # 22
TOWN = 1024
NTOK = TOWN + CC

SY = 64.0            # yT8 = y * SY
SW = 32.0            # weight scale for q/kg/v/g/out/l1/l3
SW2 = 8.0            # l2 weight scale (= glu_pre scale)
SV = 64.0            # v8 = v * SV
SOG = 256.0          # og8 = og * SOG
SP = 32.0            # score scale (mask value)
DS_P = 1.0 / (SY * SW)       # projection PSUM descale (1/2048)
DS_V = SV / (SY * SW)        # v8 = psv * (1/32)
DS_O = 1.0 / (SOG * SW)      # out-proj descale (1/8192)
DS_G1 = 1.0 / SW             # l1 PSUM descale
DS_L3 = 1.0 / (SW2 * SW)     # l3 PSUM descale (1/256)
SSTATE = 1.0 / (SP * SV)     # state/PSUM o descale (1/2048)


def _r(x):
    return np.ascontiguousarray(x)


def _bf(x):
    return np.ascontiguousarray(x.astype(ml_dtypes.bfloat16))


def _f8(x):
    return np.ascontiguousarray(
        np.clip(x, -440.0, 440.0).astype(ml_dtypes.float8_e4m3fn))


def prep_host_inputs(inputs):
    f32 = lambda k: np.asarray(inputs[k], np.float32)
    tn_w, tn_b = f32("tn_w"), f32("tn_b")
    cn_w, cn_b = f32("cn_w"), f32("cn_b")
    conv = f32("conv_w")
    convw = conv * tn_w[:, None]
    convb = tn_b * conv.sum(1)
    l1 = f32("l1_w") * cn_w[:, None]
    l2 = f32("l2_w") * cn_w[:, None]
    b1 = cn_b @ f32("l1_w")
    b2 = cn_b @ f32("l2_w")

    def tile_kxm(w, m_tiles):
        d_in, m = w.shape
        kd = d_in // P
        return w.reshape(kd, P, m_tiles, m // m_tiles).transpose(1, 0, 2, 3)

    shared = {
        "wq": _f8(tile_kxm(f32("q_w") * SW, MQ).reshape(P, -1)),
        "wkg": _f8(tile_kxm(f32("kg_w") * SW, MQ).reshape(P, -1)),
        "wv": _f8((f32("v_w") * SW).reshape(KD, P, D).transpose(1, 0, 2)
                  .reshape(P, -1)),
        "wg": _f8((f32("g_w") * SW).reshape(KD, P, D).transpose(1, 0, 2)
                  .reshape(P, -1)),
        "wout": _f8((f32("out_w") * SW).reshape(KD, P, D).transpose(1, 0, 2)
                    .reshape(P, -1)),
        "wl1": _f8((l1 * SW).reshape(KD, P, MG, P).transpose(1, 2, 0, 3)
                   .reshape(P, -1)),
        "wl2": _f8((l2 * SW2).reshape(KD, P, MG, P).transpose(1, 2, 0, 3)
                   .reshape(P, -1)),
        "wl3": _f8((f32("l3_w") * SW).reshape(MG, P, D).transpose(1, 0, 2)
                   .reshape(P, -1)),
        "convw": _r(convw.reshape(KD, P, 4).transpose(1, 0, 2).reshape(P, -1)),
        "convb": _r(convb.reshape(KD, P).T),
        "aug": _r(f32("aug_balance").reshape(MQ, P).T),
        "b1": _r(b1.reshape(MG, P).T),
        "b2": _r((b2 * SW2).reshape(MG, P).T),
    }
    x = np.asarray(inputs["x"], np.float32)

    def per_core(core):
        b, half = core // 2, core % 2
        if half == 0:
            x_seq = np.concatenate(
                [np.zeros((CC, D), np.float32), x[0:TOWN, b]], axis=0)
        else:
            x_seq = _r(x[TOWN - CC:L, b, :])
        flag = np.full((P, 1), float(half), np.float32)
        return {"x_seq": x_seq, "flag": flag, **shared}

    return per_core


def build_nc():
    nc = bacc.Bacc("TRN2", target_bir_lowering=False, debug=False)
    dt_in = {
        "x_seq": ([NTOK, D], F32),
        "flag": ([P, 1], F32),
        "wq": ([P, KD * MQ * P], F8),
        "wkg": ([P, KD * MQ * P], F8),
        "wv": ([P, KD * D], F8),
        "wg": ([P, KD * D], F8),
        "wout": ([P, KD * D], F8),
        "wl1": ([P, MG * KD * P], F8),
        "wl2": ([P, MG * KD * P], F8),
        "wl3": ([P, MG * D], F8),
        "convw": ([P, KD * 4], F32),
        "convb": ([P, KD], F32),
        "aug": ([P, MQ], F32),
        "b1": ([P, MG], F32),
        "b2": ([P, MG], F32),
    }
    dr = {k: nc.dram_tensor(k, shp, dt, kind="ExternalInput")
          for k, (shp, dt) in dt_in.items()}
    out_d = nc.dram_tensor("out", [TOWN, D], F32, kind="ExternalOutput")

    with tile.TileContext(nc) as tc:
        _emit(nc, tc, dr, out_d)
    nc.compile()
    return nc


def _emit(nc, tc, dr, out_d):
    import contextlib
    ctx = contextlib.ExitStack()
    with ctx:
        sing = ctx.enter_context(tc.tile_pool(name="sing", bufs=1))
        dram = ctx.enter_context(tc.tile_pool(name="dram", bufs=1, space="DRAM"))
        glob = ctx.enter_context(tc.tile_pool(name="glob", bufs=1))

        convw_sb = sing.tile([P, KD, 4], F32)
        nc.sync.dma_start(out=convw_sb,
                          in_=dr["convw"].ap().rearrange("p (k i) -> p k i", k=KD))
        convb_sb = sing.tile([P, KD], F32)
        nc.sync.dma_start(out=convb_sb, in_=dr["convb"].ap())
        aug_sb = sing.tile([P, MQ], F32)
        nc.sync.dma_start(out=aug_sb, in_=dr["aug"].ap())
        b1_sb = sing.tile([P, MG], F32)
        nc.sync.dma_start(out=b1_sb, in_=dr["b1"].ap())
        b2_sb = sing.tile([P, MG], F32)
        nc.sync.dma_start(out=b2_sb, in_=dr["b2"].ap())
        flag_sb = sing.tile([P, 1], F32)
        nc.sync.dma_start(out=flag_sb, in_=dr["flag"].ap())

        identb = sing.tile([P, P], BF16)
        make_identity(nc, identb)
        identf8 = sing.tile([P, P], F8)
        nc.vector.tensor_copy(out=identf8, in_=identb)
        maskc = sing.tile([P, P], BF16)
        make_upper_triangular(nc, maskc, val=SP, diag=True)
        maskc4 = sing.tile([P, 4, P], BF16)
        for j in range(4):
            nc.vector.tensor_copy(out=maskc4[:, j, :], in_=maskc)
        ind4 = sing.tile([P, 4], BF16)
        nc.vector.memset(ind4, 0.0)
        for j in range(4):
            nc.vector.memset(ind4[32 * j:32 * j + 32, j:j + 1], 1.0)
        epst = sing.tile([P, 1], F32)
        nc.vector.memset(epst, EPS)

        # glob pool: survives into P9
        zT8 = glob.tile([P, KD, TOWN], F8)
        x1_all = glob.tile([P, NCH, D], F32)

        # aug folded into block-ones reduction matrix: ind4aug[dk, g, j] =
        # aug[dk] if dk in 32-block j else 0 (used with qt*kt, since the
        # within-chunk decays of qt and kt cancel exactly)
        ind4aug = sing.tile([P, MQ, 4], BF16)
        for g in range(MQ):
            nc.vector.tensor_scalar(out=ind4aug[:, g, :], in0=ind4,
                                    scalar1=aug_sb[:, g:g + 1], scalar2=None,
                                    op0=OP.mult)

        x_ap = dr["x_seq"].ap()

        import contextlib as _ctl
        wctx = _ctl.ExitStack()
        work = wctx.enter_context(tc.tile_pool(name="work", bufs=1))
        kt = work.tile([P, MQ, TOWN], BF16)
        qt = work.tile([P, MQ, TOWN], BF16)
        dCt = work.tile([P, MQ, NCH], F32)
        dcum = work.tile([P, MQ, NCH], F32)
        s_hist = work.tile([P, NCH, MQ, 64], BF16)
        s_in = work.tile([P, MQ, 64], F32)
        wout_sb = work.tile([P, KD, D], F8)
        yT8 = work.tile([P, KD, TOWN], F8)

        # ========== P1: LN1 + transpose; P2: conv + silu -> yT8 ==========
        with tc.tile_pool(name="p12", bufs=1) as ep, \
             tc.tile_pool(name="p12ps", bufs=1, space="PSUM") as pp:
            hT = ep.tile([P, KD, NTOK], BF16)
            for t in range(NCH + 1):
                x_t = ep.tile([P, D], F32, tag="x_nat", bufs=2, name=f"x_{t}")
                nc.sync.dma_start(out=x_t, in_=x_ap[t * CC:(t + 1) * CC, :])
                bnst = ep.tile([P, 2, 6], F32, tag="bnst", bufs=2,
                               name=f"bn_{t}")
                nc.vector.bn_stats(out=bnst[:, 0, :], in_=x_t[:, 0:512])
                nc.vector.bn_stats(out=bnst[:, 1, :], in_=x_t[:, 512:1024])
                mv = ep.tile([P, 2], F32, tag="mv", bufs=2, name=f"mv_{t}")
                nc.vector.bn_aggr(out=mv, in_=bnst)
                srt = ep.tile([P, 1], F32, tag="srt", bufs=2, name=f"sq_{t}")
                nc.scalar.activation(out=srt, in_=mv[:, 1:2], func=AF.Sqrt,
                                     bias=epst[:, 0:1], scale=1.0)
                rstd = ep.tile([P, 1], F32, tag="rstd", bufs=2, name=f"rs_{t}")
                nc.vector.reciprocal(out=rstd, in_=srt)
                nmrs = ep.tile([P, 1], F32, tag="nmrs", bufs=2, name=f"nm_{t}")
                nc.vector.scalar_tensor_tensor(out=nmrs, in0=mv[:, 0:1],
                                               scalar=-1.0, in1=rstd,
                                               op0=OP.mult, op1=OP.mult)
                h = ep.tile([P, D], BF16, tag="h", bufs=2, name=f"h_{t}")
                nc.vector.tensor_scalar(out=h, in0=x_t,
                                        scalar1=rstd[:, 0:1],
                                        scalar2=nmrs[:, 0:1],
                                        op0=OP.mult, op1=OP.add)
                ptr = pp.tile([P, D], BF16, tag="ptr", bufs=2, name=f"ptr_{t}")
                for dt_i in range(KD):
                    nc.tensor.transpose(ptr[:, dt_i * P:(dt_i + 1) * P],
                                        h[:, dt_i * P:(dt_i + 1) * P], identb)
                nc.scalar.copy(out=hT[:, :, t * CC:(t + 1) * CC],
                               in_=ptr.rearrange("p (k c) -> p k c", k=KD))

            nc.sync.dma_start(
                out=wout_sb,
                in_=dr["wout"].ap().rearrange("p (k n) -> p k n", k=KD))

            # conv on PE: diagonal weight tiles, 4 taps accumulate in PSUM
            cdiag = ep.tile([P, KD, 4, P], BF16)
            for dt_i in range(KD):
                for i in range(4):
                    nc.vector.tensor_scalar(out=cdiag[:, dt_i, i, :],
                                            in0=identb,
                                            scalar1=convw_sb[:, dt_i, i:i + 1],
                                            scalar2=None, op0=OP.mult)
            for dt_i in range(KD):
                pc = pp.tile([P, TOWN], F32, tag="pc", bufs=2,
                             name=f"pc_{dt_i}")
                for n in range(2):
                    nsl = slice(n * 512, (n + 1) * 512)
                    for i in range(4):
                        nc.tensor.matmul(
                            pc[:, nsl], cdiag[:, dt_i, i, :],
                            hT[:, dt_i, 125 + i + n * 512:
                               125 + i + n * 512 + 512],
                            start=(i == 0), stop=(i == 3))
                yt_t = ep.tile([P, TOWN], BF16, tag="yt", bufs=2,
                               name=f"yt_{dt_i}")
                nc.scalar.activation(out=yt_t, in_=pc, func=AF.Silu,
                                     bias=convb_sb[:, dt_i:dt_i + 1], scale=1.0)
                nc.vector.tensor_scalar(out=yT8[:, dt_i, :], in0=yt_t,
                                        scalar1=SY, scalar2=None, op0=OP.mult)

        # mid pool: v8/g_t/sg + v/g weights (preloaded NOW, during P3)
        mid = wctx.enter_context(tc.tile_pool(name="mid", bufs=1))
        wv_sb = mid.tile([P, KD, D], F8)
        nc.sync.dma_start(
            out=wv_sb, in_=dr["wv"].ap().rearrange("p (k n) -> p k n", k=KD))
        wg_sb = mid.tile([P, KD, D], F8)
        nc.sync.dma_start(
            out=wg_sb, in_=dr["wg"].ap().rearrange("p (k n) -> p k n", k=KD))
        v8 = mid.tile([P, NCH, D], F8)
        g_t = mid.tile([P, NCH, D], BF16)
        sg = mid.tile([P, NCH, D], BF16)

        # ===== P3: kg/q projections + gating; v-projs fill the V-chain =====
        with tc.tile_pool(name="p3", bufs=1) as e3, \
             tc.tile_pool(name="p3ps", bufs=1, space="PSUM") as pp3:
            wq_sb = e3.tile([P, KD, MQ, P], F8)
            nc.sync.dma_start(
                out=wq_sb,
                in_=dr["wq"].ap().rearrange("p (k m c) -> p k m c", k=KD, m=MQ))
            wkg_sb = e3.tile([P, KD, MQ, P], F8)
            nc.sync.dma_start(
                out=wkg_sb,
                in_=dr["wkg"].ap().rearrange("p (k m c) -> p k m c", k=KD, m=MQ))

            def emit_vproj(c):
                csl = slice(c * CC, (c + 1) * CC)
                psv = pp3.tile([P, D], F32, tag="psv", bufs=2, name=f"psv_{c}")
                for kp in range(KD // 2):
                    ksl = slice(2 * kp, 2 * kp + 2)
                    for n in range(2):
                        nsl = slice(n * 512, (n + 1) * 512)
                        nc.tensor.matmul(psv[:, nsl], yT8[:, ksl, csl],
                                         wv_sb[:, ksl, nsl], start=(kp == 0),
                                         stop=(kp == KD // 2 - 1),
                                         perf_mode=DRM)
                nc.scalar.mul(out=v8[:, c, :], in_=psv, mul=DS_V)

            for g0 in range(0, NG, 2):
                qTb, sps, As = {}, {}, {}
                for g in (g0, g0 + 1):
                    ps = pp3.tile([P, TOWN], F32, tag="pproj", bufs=2,
                                  name=f"pkg_{g}")
                    for kp in range(KD // 2):
                        ksl = slice(2 * kp, 2 * kp + 2)
                        for n in range(2):
                            nsl = slice(n * 512, (n + 1) * 512)
                            nc.tensor.matmul(ps[:, nsl],
                                             wkg_sb[:, ksl, g, :],
                                             yT8[:, ksl, nsl],
                                             start=(kp == 0),
                                             stop=(kp == KD // 2 - 1),
                                             perf_mode=DRM)
                    t0 = e3.tile([P, TOWN], BF16, tag="t0", bufs=2,
                                 name=f"t0_{g}")
                    nc.scalar.activation(out=t0, in_=ps, func=AF.Exp,
                                         scale=-DS_P)
                    sp = e3.tile([P, TOWN], F32, tag="sp", bufs=2,
                                 name=f"sp_{g}")
                    nc.scalar.activation(out=sp, in_=t0, func=AF.Ln,
                                         bias=1.0, scale=1.0)
                    sps[g] = sp
                    ps2 = pp3.tile([P, TOWN], F32, tag="pproj", bufs=2,
                                   name=f"pq_{g}")
                    for kp in range(KD // 2):
                        ksl = slice(2 * kp, 2 * kp + 2)
                        for n in range(2):
                            nsl = slice(n * 512, (n + 1) * 512)
                            nc.tensor.matmul(ps2[:, nsl],
                                             wq_sb[:, ksl, g, :],
                                             yT8[:, ksl, nsl],
                                             start=(kp == 0),
                                             stop=(kp == KD // 2 - 1),
                                             perf_mode=DRM)
                    q_g = e3.tile([P, TOWN], BF16, tag="qTb", bufs=2,
                                  name=f"qTb_{g}")
                    nc.scalar.mul(out=q_g, in_=ps2, mul=DS_P)
                    qTb[g] = q_g
                # v-projections: PE work that overlaps this pair's V chain
                for c in (range(0, 4) if g0 == 0 else range(4, NCH)):
                    emit_vproj(c)
                # scan + chunk rebase (Vector)
                for g in (g0, g0 + 1):
                    A = e3.tile([P, TOWN], F32, tag="A", bufs=2, name=f"A_{g}")
                    nc.vector.tensor_tensor_scan(out=A, data0=sps[g],
                                                 data1=sps[g], initial=0.0,
                                                 op0=OP.add, op1=OP.bypass)
                    As[g] = A
                    bx = e3.tile([P, NCH + 1], F32, tag="bx", bufs=2,
                                 name=f"bx_{g}")
                    nc.vector.memset(bx[:, 0:1], 0.0)
                    nc.vector.tensor_scalar(
                        out=bx[:, 1:NCH + 1],
                        in0=A.rearrange("p (c t) -> p c t", c=NCH)[:, :, CC - 1],
                        scalar1=1.0 / NORM, scalar2=None, op0=OP.mult)
                    nc.scalar.activation(out=dcum[:, g, :],
                                         in_=bx[:, 0:NCH],
                                         func=AF.Exp, scale=-1.0)
                    nbx = e3.tile([P, NCH], F32, tag="nbx", bufs=2,
                                  name=f"nbx_{g}")
                    nc.vector.tensor_scalar(out=nbx, in0=bx[:, 0:NCH],
                                            scalar1=-NORM, scalar2=None,
                                            op0=OP.mult)
                    for c in range(1, NCH):
                        nc.vector.tensor_scalar(
                            out=A[:, c * CC:(c + 1) * CC],
                            in0=A[:, c * CC:(c + 1) * CC],
                            scalar1=nbx[:, c:c + 1], scalar2=None, op0=OP.add)
                for g in (g0, g0 + 1):
                    ek = e3.tile([P, TOWN], F32, tag="ek", bufs=2,
                                 name=f"ek_{g}")
                    nc.scalar.activation(out=ek, in_=sps[g], func=AF.Exp,
                                         scale=-1.0 / NORM)
                    eA = e3.tile([P, TOWN], BF16, tag="eA", bufs=2,
                                 name=f"eA_{g}")
                    nc.scalar.activation(out=eA, in_=As[g], func=AF.Exp,
                                         scale=-1.0 / NORM)
                    erA = e3.tile([P, TOWN], BF16, tag="erA", bufs=2,
                                  name=f"erA_{g}")
                    nc.scalar.activation(out=erA, in_=As[g], func=AF.Exp,
                                         scale=1.0 / NORM)
                    nc.scalar.activation(
                        out=dCt[:, g, :],
                        in_=As[g].rearrange("p (c t) -> p c t",
                                            c=NCH)[:, :, CC - 1],
                        func=AF.Exp, scale=-1.0 / NORM)
                    k_ = e3.tile([P, TOWN], BF16, tag="k_", bufs=2,
                                 name=f"k__{g}")
                    nc.vector.tensor_scalar(out=k_, in0=ek, scalar1=-1.0,
                                            scalar2=1.0, op0=OP.mult,
                                            op1=OP.add)
                    nc.vector.tensor_tensor(out=kt[:, g, :], in0=k_, in1=erA,
                                            op=OP.mult)
                    nc.gpsimd.tensor_tensor(out=qt[:, g, :], in0=qTb[g],
                                            in1=eA, op=OP.mult)

        # ===== P5: state scan interleaved with g-projection; collective =====
        s_out_b = dram.tile([P, MQ * 64], F32)
        s_gth_b = dram.tile([2, P, MQ * 64], F32)
        with tc.tile_pool(name="p45", bufs=1) as e5, \
             tc.tile_pool(name="p45ps", bufs=1, space="PSUM") as pp5:
            s_st = e5.tile([P, MQ, 64], F32)
            nc.vector.memset(s_st, 0.0)
            for c in range(NCH):
                csl = slice(c * CC, (c + 1) * CC)
                nc.gpsimd.tensor_copy(out=s_hist[:, c, :, :], in_=s_st)
                for g in range(NG):
                    ktmp = e5.tile([P, P], BF16, tag="ktmp", bufs=4,
                                   name=f"ktmp_{c}_{g}")
                    nc.vector.tensor_scalar(out=ktmp, in0=kt[:, g, csl],
                                            scalar1=dCt[:, g, c:c + 1],
                                            scalar2=SP, op0=OP.mult,
                                            op1=OP.mult)
                    ktr = pp5.tile([P, 1024], BF16, tag="ktr", bufs=2,
                                   name=f"ktr_{c}_{g}")
                    nc.tensor.transpose(ktr[:, 0:P], ktmp, identb)
                    kn = e5.tile([P, P], F8, tag="kn", bufs=4,
                                 name=f"kn_{c}_{g}")
                    nc.scalar.copy(out=kn, in_=ktr[:, 0:P])
                    dsp = pp5.tile([P, 512], F32, tag="dsp", bufs=2,
                                   name=f"dsp_{c}_{g}")
                    for hh in range(4):
                        hd = 4 * g + hh
                        hs = slice(32 * hh, 32 * hh + 32)
                        nc.tensor.matmul(dsp[hs, 0:64], kn[:, hs],
                                         v8[:, c, 64 * hd:64 * hd + 64],
                                         start=True, stop=True,
                                         tile_position=(0, 32 * hh))
                    nc.vector.scalar_tensor_tensor(out=s_st[:, g, :],
                                                   in0=s_st[:, g, :],
                                                   scalar=dCt[:, g, c:c + 1],
                                                   in1=dsp[:, 0:64],
                                                   op0=OP.mult, op1=OP.add)
                if c == NCH - 1:
                    nc.gpsimd.dma_start(out=s_out_b,
                                        in_=s_st.rearrange("p g c -> p (g c)"))
                    nc.gpsimd.collective_compute(
                        "AllGather", OP.bypass,
                        replica_groups=[[0, 1], [2, 3], [4, 5], [6, 7]],
                        ins=[s_out_b.opt()], outs=[s_gth_b.opt()],
                    )
                # g projection for chunk c (PE work hiding the scan chain)
                psg = pp5.tile([P, D], F32, tag="psg", bufs=1, name=f"psg_{c}")
                for kp in range(KD // 2):
                    ksl = slice(2 * kp, 2 * kp + 2)
                    for n in range(2):
                        nsl = slice(n * 512, (n + 1) * 512)
                        nc.tensor.matmul(psg[:, nsl], yT8[:, ksl, csl],
                                         wg_sb[:, ksl, nsl], start=(kp == 0),
                                         stop=(kp == KD // 2 - 1),
                                         perf_mode=DRM)
                nc.scalar.activation(out=g_t[:, c, :], in_=psg, func=AF.Silu,
                                     scale=DS_P)

        # ===== P7: aug + scores + o-assembly + gn, one pipelined loop =====
        with tc.tile_pool(name="p7", bufs=1) as e7, \
             tc.tile_pool(name="p7ps", bufs=1, space="PSUM") as pp7:
            o8s, ptms, og8s, xrs = {}, {}, {}, {}
            srtos, mos, o8s2 = {}, {}, {}
            rstd2s, nmrs2s = {}, {}

            def emit_prep(c):
                # aug term (qt*kt = q*k: within-chunk decays cancel)
                csl = slice(c * CC, (c + 1) * CC)
                qkc = e7.tile([P, MQ, P], BF16, tag="qkc", bufs=2,
                              name=f"qkc_{c}")
                for g in range(NG):
                    nc.gpsimd.tensor_tensor(out=qkc[:, g, :],
                                            in0=qt[:, g, csl],
                                            in1=kt[:, g, csl], op=OP.mult)
                pa = pp7.tile([P, 512], F32, tag="psaug", bufs=1,
                              name=f"pa_{c}")
                for g in range(NG):
                    nc.tensor.matmul(pa[:, 4 * g:4 * g + 4], qkc[:, g, :],
                                     ind4aug[:, g, :], start=True, stop=True)
                aug_t = e7.tile([P, H], BF16, tag="aug_t", bufs=2,
                                name=f"at_{c}")
                nc.vector.tensor_scalar(out=aug_t, in0=pa[:, 0:H],
                                        scalar1=1.0 / SV,
                                        scalar2=None, op0=OP.mult)
                augv = e7.tile([P, D], BF16, tag="augv", bufs=2,
                               name=f"av_{c}")
                nc.vector.tensor_tensor(
                    out=augv.rearrange("p (h c) -> p h c", h=H),
                    in0=v8[:, c, :].rearrange("p (h c) -> p h c", h=H),
                    in1=aug_t[:, :, None].broadcast_to((P, H, 64)), op=OP.mult)
                nc.scalar.activation(out=sg[:, c, :], in_=augv,
                                     func=AF.Sigmoid)
                # masked intra-chunk attention scores
                ptmc = e7.tile([P, NG, 4, P], F8, tag="ptm", bufs=5,
                               name=f"ptm_{c}")
                for g in range(NG):
                    for hh in range(4):
                        hs = slice(32 * hh, 32 * hh + 32)
                        pt_ps = pp7.tile([P, 512], F32, tag="ptps", bufs=2,
                                         name=f"ptps_{c}_{g}_{hh}")
                        nc.tensor.matmul(pt_ps[:, 0:P], kt[hs, g, csl],
                                         qt[hs, g, csl],
                                         start=True, stop=True,
                                         tile_position=(32 * hh, 0))
                        nc.vector.tensor_tensor(out=ptmc[:, g, hh, :],
                                                in0=pt_ps[:, 0:P], in1=maskc,
                                                op=OP.mult)
                ptms[c] = ptmc

            def emit_oass(c):
                csl = slice(c * CC, (c + 1) * CC)
                ptmc = ptms.pop(c)
                se1 = e7.tile([P, MQ, 64], F32, tag="se1", bufs=2,
                              name=f"se1_{c}")
                nc.vector.tensor_tensor(
                    out=se1, in0=s_in,
                    in1=dcum[:, :, c, None].broadcast_to((P, MQ, 64)),
                    op=OP.mult)
                s_eff = e7.tile([P, MQ, 64], BF16, tag="seff", bufs=2,
                                name=f"seff_{c}")
                nc.vector.tensor_tensor(out=s_eff, in0=se1,
                                        in1=s_hist[:, c, :, :], op=OP.add)
                o_ps = pp7.tile([P, D], F32, tag="pso", bufs=2,
                                name=f"ops_{c}")
                for g in range(NG):
                    for hh in range(4):
                        hd = 4 * g + hh
                        hs = slice(32 * hh, 32 * hh + 32)
                        osl = slice(64 * hd, 64 * hd + 64)
                        nc.tensor.matmul(o_ps[:, osl], qt[hs, g, csl],
                                         s_eff[hs, g, :], start=True,
                                         stop=False,
                                         tile_position=(32 * hh, 0))
                        nc.tensor.matmul(o_ps[:, osl], ptmc[:, g, hh, :],
                                         v8[:, c, osl], start=False, stop=True)
                o8 = e7.tile([P, D], BF16, tag="o8", bufs=4, name=f"o8_{c}")
                nc.vector.scalar_tensor_tensor(out=o8, in0=o_ps,
                                               scalar=SSTATE, in1=sg[:, c, :],
                                               op0=OP.mult, op1=OP.add)
                o8s[c] = o8

            def emit_gn(c):
                csl = slice(c * CC, (c + 1) * CC)
                o8 = o8s.pop(c)
                o_h = o8.rearrange("p (h c) -> p h c", h=H)
                x_res = e7.tile([P, D], F32, tag="x_res", bufs=3,
                                name=f"xr_{c}")
                nc.sync.dma_start(out=x_res,
                                  in_=x_ap[CC + c * CC:CC + (c + 1) * CC, :])
                xrs[c] = x_res
                osq = e7.tile([P, D], BF16, tag="osq", bufs=2, name=f"osq_{c}")
                nc.vector.tensor_tensor(out=osq, in0=o8, in1=o8, op=OP.mult)
                s1 = e7.tile([P, H], F32, tag="s1", bufs=2, name=f"s1_{c}")
                nc.vector.tensor_reduce(out=s1, in_=o_h,
                                        axis=mybir.AxisListType.X, op=OP.add)
                s2 = e7.tile([P, H], F32, tag="s2", bufs=2, name=f"s2_{c}")
                nc.vector.tensor_reduce(
                    out=s2, in_=osq.rearrange("p (h c) -> p h c", h=H),
                    axis=mybir.AxisListType.X, op=OP.add)
                mo = e7.tile([P, H], F32, tag="mo", bufs=2, name=f"mo_{c}")
                nc.vector.tensor_scalar(out=mo, in0=s1, scalar1=1.0 / 64.0,
                                        scalar2=None, op0=OP.mult)
                mo2 = e7.tile([P, H], F32, tag="mo2", bufs=2, name=f"mo2_{c}")
                nc.vector.tensor_tensor(out=mo2, in0=mo, in1=mo, op=OP.mult)
                varo = e7.tile([P, H], F32, tag="varo", bufs=2,
                               name=f"varo_{c}")
                nc.vector.scalar_tensor_tensor(out=varo, in0=s2,
                                               scalar=1.0 / 64.0, in1=mo2,
                                               op0=OP.mult, op1=OP.subtract)
                srto = e7.tile([P, H], F32, tag="srto", bufs=3,
                               name=f"sqo_{c}")
                nc.scalar.activation(out=srto, in_=varo, func=AF.Sqrt,
                                     bias=epst[:, 0:1], scale=1.0)
                srtos[c] = srto
                mos[c] = mo
                o8s2[c] = o8

            def emit_gn_a2(c):
                srto = srtos.pop(c)
                mo = mos.pop(c)
                o8 = o8s2.pop(c)
                o_h = o8.rearrange("p (h c) -> p h c", h=H)
                rstdo = e7.tile([P, H], F32, tag="rstdo", bufs=2,
                                name=f"rso_{c}")
                nc.vector.reciprocal(out=rstdo, in_=srto)
                nmo = e7.tile([P, H], BF16, tag="nmo", bufs=2, name=f"nmo_{c}")
                nc.vector.scalar_tensor_tensor(out=nmo, in0=mo, scalar=-1.0,
                                               in1=rstdo, op0=OP.mult,
                                               op1=OP.mult)
                on1 = e7.tile([P, D], BF16, tag="on1", bufs=2, name=f"on1_{c}")
                nc.vector.tensor_tensor(
                    out=on1.rearrange("p (h c) -> p h c", h=H), in0=o_h,
                    in1=rstdo[:, :, None].broadcast_to((P, H, 64)), op=OP.mult)
                on2 = e7.tile([P, D], BF16, tag="on2", bufs=2, name=f"on2_{c}")
                nc.vector.tensor_tensor(
                    out=on2.rearrange("p (h c) -> p h c", h=H),
                    in0=on1.rearrange("p (h c) -> p h c", h=H),
                    in1=nmo[:, :, None].broadcast_to((P, H, 64)), op=OP.add)
                og8 = e7.tile([P, D], BF16, tag="og8", bufs=3,
                              name=f"og_{c}")
                nc.vector.scalar_tensor_tensor(out=og8, in0=on2, scalar=SOG,
                                               in1=g_t[:, c, :], op0=OP.mult,
                                               op1=OP.mult)
                og8s[c] = og8

            def emit_gn_b(c):
                csl = slice(c * CC, (c + 1) * CC)
                og8 = og8s.pop(c)
                x_res = xrs.pop(c)
                ogt_ps = pp7.tile([P, D], BF16, tag="trps", bufs=1,
                                  name=f"ogt_{c}")
                for dt_i in range(KD):
                    nc.tensor.transpose(ogt_ps[:, dt_i * P:(dt_i + 1) * P],
                                        og8[:, dt_i * P:(dt_i + 1) * P],
                                        identb)
                ogT8 = e7.tile([P, KD, P], F8, tag="ogT", bufs=2,
                               name=f"ogT_{c}")
                nc.scalar.copy(out=ogT8,
                               in_=ogt_ps.rearrange("p (k c) -> p k c", k=KD))
                out_ps = pp7.tile([P, D], F32, tag="pso", bufs=2,
                                  name=f"op_{c}")
                for kp in range(KD // 2):
                    ksl = slice(2 * kp, 2 * kp + 2)
                    for n in range(2):
                        nsl = slice(n * 512, (n + 1) * 512)
                        nc.tensor.matmul(out_ps[:, nsl], ogT8[:, ksl, :],
                                         wout_sb[:, ksl, nsl],
                                         start=(kp == 0),
                                         stop=(kp == KD // 2 - 1),
                                         perf_mode=DRM)
                nc.vector.scalar_tensor_tensor(out=x1_all[:, c, :],
                                               in0=out_ps, scalar=DS_O,
                                               in1=x_res, op0=OP.mult,
                                               op1=OP.add)
                # LN2
                bn2 = e7.tile([P, 2, 6], F32, tag="bn2", bufs=2,
                              name=f"bn2_{c}")
                nc.vector.bn_stats(out=bn2[:, 0, :], in_=x1_all[:, c, 0:512])
                nc.vector.bn_stats(out=bn2[:, 1, :], in_=x1_all[:, c, 512:1024])
                mv2 = e7.tile([P, 2], F32, tag="mv2", bufs=2, name=f"mv2_{c}")
                nc.vector.bn_aggr(out=mv2, in_=bn2)
                srt2 = e7.tile([P, 1], F32, tag="srt2", bufs=2,
                               name=f"sq2_{c}")
                nc.scalar.activation(out=srt2, in_=mv2[:, 1:2], func=AF.Sqrt,
                                     bias=epst[:, 0:1], scale=1.0)
                rstd2 = e7.tile([P, 1], F32, tag="rstd2", bufs=3,
                                name=f"rs2_{c}")
                nc.vector.reciprocal(out=rstd2, in_=srt2)
                nmrs2 = e7.tile([P, 1], F32, tag="nmrs2", bufs=3,
                                name=f"nm2_{c}")
                nc.vector.scalar_tensor_tensor(out=nmrs2, in0=mv2[:, 0:1],
                                               scalar=-1.0, in1=rstd2,
                                               op0=OP.mult, op1=OP.mult)
                rstd2s[c] = rstd2
                nmrs2s[c] = nmrs2

            def emit_gn_c(c):
                csl = slice(c * CC, (c + 1) * CC)
                rstd2 = rstd2s.pop(c)
                nmrs2 = nmrs2s.pop(c)
                z8 = e7.tile([P, D], BF16, tag="z8", bufs=2, name=f"z_{c}")
                nc.scalar.activation(out=z8, in_=x1_all[:, c, :],
                                     func=AF.Identity,
                                     bias=nmrs2[:, 0:1], scale=rstd2[:, 0:1])
                ztr_ps = pp7.tile([P, D], BF16, tag="trps", bufs=1,
                                  name=f"ztr_{c}")
                for dt_i in range(KD):
                    nc.tensor.transpose(ztr_ps[:, dt_i * P:(dt_i + 1) * P],
                                        z8[:, dt_i * P:(dt_i + 1) * P],
                                        identb)
                nc.scalar.copy(out=zT8[:, :, csl],
                               in_=ztr_ps.rearrange("p (k c) -> p k c", k=KD))

            s_in_raw = e7.tile([P, MQ * 64], F32)
            nc.gpsimd.dma_start(out=s_in_raw, in_=s_gth_b[0])
            for c in range(6):
                emit_prep(c)
            # flag-mult emitted after 6 chunks of prep cover (~27us)
            nc.vector.tensor_scalar(out=s_in.rearrange("p g c -> p (g c)"),
                                    in0=s_in_raw, scalar1=flag_sb[:, 0:1],
                                    scalar2=None, op0=OP.mult)
            for c in range(NCH):
                emit_oass(c)
                if c + 6 < NCH:
                    emit_prep(c + 6)
                if c >= 1:
                    emit_gn(c - 1)
                if c >= 2:
                    emit_gn_a2(c - 2)
                if c >= 3:
                    emit_gn_b(c - 3)
                if c >= 4:
                    emit_gn_c(c - 4)
            emit_gn(NCH - 1)
            emit_gn_a2(NCH - 2)
            emit_gn_b(NCH - 3)
            emit_gn_c(NCH - 4)
            emit_gn_a2(NCH - 1)
            emit_gn_b(NCH - 2)
            emit_gn_c(NCH - 3)
            emit_gn_b(NCH - 1)
            emit_gn_c(NCH - 2)
            emit_gn_c(NCH - 1)

        # close work/mid pools before GLU
        wctx.close()

        # ========== P9: GLU ==========
        wl1_v = dr["wl1"].ap().rearrange("p (m k c) -> p m k c", m=MG, k=KD)
        wl2_v = dr["wl2"].ap().rearrange("p (m k c) -> p m k c", m=MG, k=KD)
        with tc.tile_pool(name="gl", bufs=1) as gl:
            wl3_sb = gl.tile([P, MG, D], F8)
            nc.sync.dma_start(
                out=wl3_sb,
                in_=dr["wl3"].ap().rearrange("p (k n) -> p k n", k=MG))
            gluT8 = gl.tile([P, MG, TOWN], F8)
            with tc.tile_pool(name="glps", bufs=1, space="PSUM") as glps:
                for m in range(MG):
                    l1m = gl.tile([P, KD, P], F8, tag="l1m", bufs=3,
                                  name=f"l1m_{m}")
                    nc.sync.dma_start(out=l1m, in_=wl1_v[:, m, :, :])
                    l2m = gl.tile([P, KD, P], F8, tag="l2m", bufs=3,
                                  name=f"l2m_{m}")
                    nc.sync.dma_start(out=l2m, in_=wl2_v[:, m, :, :])
                    g1p = glps.tile([P, TOWN], F32, tag="psg1", bufs=2,
                                    name=f"g1p_{m}")
                    for kp in range(KD // 2):
                        ksl = slice(2 * kp, 2 * kp + 2)
                        for n in range(2):
                            nsl = slice(n * 512, (n + 1) * 512)
                            nc.tensor.matmul(g1p[:, nsl], l1m[:, ksl, :],
                                             zT8[:, ksl, nsl], start=(kp == 0),
                                             stop=(kp == KD // 2 - 1),
                                             perf_mode=DRM)
                    g1s = gl.tile([P, TOWN], BF16, tag="g1s", bufs=2,
                                  name=f"g1s_{m}")
                    nc.scalar.activation(out=g1s, in_=g1p, func=AF.Silu,
                                         bias=b1_sb[:, m:m + 1], scale=DS_G1)
                    g2p = glps.tile([P, TOWN], F32, tag="psg2", bufs=2,
                                    name=f"g2p_{m}")
                    for kp in range(KD // 2):
                        ksl = slice(2 * kp, 2 * kp + 2)
                        for n in range(2):
                            nsl = slice(n * 512, (n + 1) * 512)
                            nc.tensor.matmul(g2p[:, nsl], l2m[:, ksl, :],
                                             zT8[:, ksl, nsl], start=(kp == 0),
                                             stop=(kp == KD // 2 - 1),
                                             perf_mode=DRM)
                    nc.vector.scalar_tensor_tensor(out=gluT8[:, m, :], in0=g2p,
                                                   scalar=b2_sb[:, m:m + 1],
                                                   in1=g1s, op0=OP.add,
                                                   op1=OP.mult)

            with tc.tile_pool(name="g3", bufs=1) as g3, \
                 tc.tile_pool(name="g3ps", bufs=1, space="PSUM") as g3ps:
                for t in range(2):
                    psf = [g3ps.tile([P, 512], F32, tag="psf", bufs=8,
                                     name=f"psf_{t}_{i}") for i in range(8)]
                    for kp in range(MG // 2):
                        ksl = slice(2 * kp, 2 * kp + 2)
                        for cc in range(4):
                            tsl = slice(t * 512 + cc * P, t * 512 + (cc + 1) * P)
                            for n in range(2):
                                nsl = slice(n * 512, (n + 1) * 512)
                                nc.tensor.matmul(
                                    psf[cc * 2 + n], gluT8[:, ksl, tsl],
                                    wl3_sb[:, ksl, nsl],
                                    start=(kp == 0), stop=(kp == MG // 2 - 1),
                                    perf_mode=DRM)
                    for cc in range(4):
                        ch = t * 4 + cc
                        trow = t * 512 + cc * CC
                        outc = g3.tile([P, D], F32, tag="outc", bufs=2,
                                       name=f"outc_{t}_{cc}")
                        for n in range(2):
                            nc.vector.scalar_tensor_tensor(
                                out=outc[:, n * 512:(n + 1) * 512],
                                in0=psf[cc * 2 + n], scalar=DS_L3,
                                in1=x1_all[:, ch, n * 512:(n + 1) * 512],
                                op0=OP.mult, op1=OP.add)
                        nc.sync.dma_start(out=out_d.ap()[trow:trow + CC, :],
                                          in_=outc)


_NC_CACHE = {}


def get_nc():
    if "nc" not in _NC_CACHE:
        _NC_CACHE["nc"] = build_nc()
    return _NC_CACHE["nc"]


def kernel(**inputs):
    nc = get_nc()
    per_core = prep_host_inputs(inputs)
    in_maps = [per_core(c) for c in range(8)]
    res = run_bass_kernel_spmd(nc, in_maps, core_ids=list(range(8)))
    out = np.zeros((L, B, D), np.float32)
    for c in range(8):
        b, half = c // 2, c % 2
        out[half * TOWN:(half + 1) * TOWN, b, :] = res.results[c]["out"]
    return out
